# revision 31
# baseline (speedup 1.0000x reference)
"""EnforceDecrease kernel for Trainium2 (8 NeuronCores, data-parallel over N).

Computation (per waveform n, channel i):
    ptp[n,i]   = max_t wf[n,t,i] - min_t wf[n,t,i]
    pmin[n,i]  = min over parents p of padded ptp[n, parents_index[mc[n],i,p]]
                 (index c==40 is an +inf sentinel)
    resc[n,i]  = min(pmin/ptp, 1)
    out_wf     = wf * resc;  out_ptp = ptp * resc

Device mapping: partition = waveform (tpw=128 waveforms per tile), DVE does
the T-axis min/max reductions and the masked parent-min reduce, the masked
add (mask u8 * 1e30 + ptp broadcast) is split DVE/GPSIMD for engine balance,
GPSIMD does the big broadcast multiply, HWDGE DMAs stream waveforms in/out,
and per-waveform mask rows are gathered up-front with indirect DMA.
"""

import contextlib

import numpy as np

N, T, C = 8192, 121, 40  # waveforms, timesteps, channels
CPAR, PPAR = 384, 12  # parents table dims
NCORES = 8
NSH = N // NCORES  # waveforms per core
D = T * C
BIG = 1e30

# tiles whose masked-add runs on gpsimd (rest on DVE): engine balance
STT_ON_POOL = 5

_prog_cache: dict = {}


def _build_program(n_sh=NSH, tpw=128, reps=1, variant="full"):
    """Build + compile the per-core Bass program. tpw = waveforms per tile.
    reps > 1 wraps the tile loop in a hardware loop (benchmarking only).
    variant: "full" | "dma" (stream only) | "novmul" (skip pool multiply)."""
    import concourse.bass as bass
    import concourse.tile as tile
    from concourse import bacc, mybir

    dt = mybir.dt
    AX = mybir.AxisListType
    OP = mybir.AluOpType

    slots = tpw // 128  # waveforms per partition per tile
    ntiles = n_sh // tpw
    nslots = ntiles * slots
    mrow = C * C  # mask row elements per waveform

    nc = bacc.Bacc("TRN2", target_bir_lowering=False, debug=False)
    wf_in = nc.dram_tensor("wf", (n_sh, D), dt.float32, kind="ExternalInput").ap()
    mask_table = nc.dram_tensor(
        "mask_table", (CPAR, mrow), dt.bfloat16, kind="ExternalInput"
    ).ap()
    idx = nc.dram_tensor("idx", (n_sh,), dt.int32, kind="ExternalInput").ap()
    wf_out = nc.dram_tensor("wf_out", (n_sh, D), dt.float32, kind="ExternalOutput").ap()
    ptps_out = nc.dram_tensor(
        "ptps_out", (n_sh, C), dt.float32, kind="ExternalOutput"
    ).ap()

    wf_bufs = 7 if slots == 1 else 3
    with tile.TileContext(nc) as tc:
        with (
            tc.tile_pool(name="wfp", bufs=wf_bufs) as wf_pool,
            tc.tile_pool(name="masks", bufs=1) as masks_pool,
            tc.tile_pool(name="tmp", bufs=2 if slots > 1 else 3) as tmp_pool,
            tc.tile_pool(name="stats", bufs=6 if slots == 1 else 3) as stats_pool,
        ):
            # All per-waveform mask rows gathered up-front (gpsimd/SWDGE).
            idx_t = masks_pool.tile([128, ntiles, slots], dt.int32)
            nc.sync.dma_start(
                idx_t[:], idx.rearrange("(t p k) -> p t k", t=ntiles, k=slots)
            )
            masks = masks_pool.tile([128, nslots * mrow], dt.bfloat16)
            ptpsx_all = masks_pool.tile([128, nslots, C], dt.float32)
            for t in range(ntiles):
                for k in range(slots):
                    s = t * slots + k
                    nc.gpsimd.indirect_dma_start(
                        out=masks[:, s * mrow : (s + 1) * mrow],
                        out_offset=None,
                        in_=mask_table,
                        in_offset=bass.IndirectOffsetOnAxis(
                            ap=idx_t[:, t, k : k + 1], axis=0
                        ),
                    )

            rep_ctx = tc.For_i(0, reps, 1) if reps > 1 else contextlib.nullcontext()
            with rep_ctx:
                for t in range(ntiles):
                    sl = slice(t * tpw, (t + 1) * tpw)
                    wf = wf_pool.tile([128, slots * D], dt.float32)
                    nc.sync.dma_start(
                        wf[:], wf_in[sl, :].rearrange("(p k) d -> p (k d)", k=slots)
                    )
                    if variant == "dma":
                        nc.sync.dma_start(
                            wf_out[sl, :].rearrange("(p k) d -> p (k d)", k=slots),
                            wf[:],
                        )
                        continue
                    wf4 = wf[:].rearrange("p (k t c) -> p k t c", k=slots, t=T, c=C)
                    wfT = wf4.transpose([0, 1, 3, 2])  # [128, slots, C, T]

                    mx = stats_pool.tile([128, slots, C], dt.float32, tag="mx")
                    mn = stats_pool.tile([128, slots, C], dt.float32, tag="mn")
                    ptps = stats_pool.tile([128, slots, C], dt.float32, tag="ptps")
                    if variant == "nored":
                        # fake ptps: cheap copy of the first C elements
                        nc.vector.tensor_scalar_max(
                            ptps[:], wf[:].rearrange("p (k d) -> p k d", k=slots)[:, :, :C], 0.01
                        )
                    else:
                        nc.vector.tensor_reduce(mx[:], wfT, axis=AX.X, op=OP.max)
                        nc.vector.tensor_reduce(mn[:], wfT, axis=AX.X, op=OP.min)
                        nc.vector.tensor_sub(ptps[:], mx[:], mn[:])
                    if variant == "reduces":
                        nc.sync.dma_start(
                            wf_out[sl, :].rearrange("(p k) d -> p (k d)", k=slots),
                            wf[:],
                        )
                        continue

                    # masked[p,k,i,j] = mask{0,BIG} + ptp[p,k,j]; then min over j
                    # (TensorTensor APs must be <=3D: one op per slot)
                    masked = tmp_pool.tile([128, slots, C, C], dt.float32, tag="masked")
                    mview = masks[
                        :, t * slots * mrow : (t + 1) * slots * mrow
                    ].rearrange("p (k i j) -> p k i j", k=slots, i=C, j=C)
                    stt_eng = nc.gpsimd if t < STT_ON_POOL else nc.vector
                    for k in range(slots):
                        stt_eng.tensor_tensor(
                            out=masked[:, k],
                            in0=mview[:, k],
                            in1=ptps[:][:, k, None, :].to_broadcast([128, C, C]),
                            op=OP.add,
                        )
                    pmin = stats_pool.tile([128, slots, C], dt.float32, tag="pmin")
                    nc.vector.tensor_reduce(pmin[:], masked[:], axis=AX.X, op=OP.min)

                    rcp = stats_pool.tile([128, slots, C], dt.float32, tag="rcp")
                    nc.vector.reciprocal(rcp[:], ptps[:])
                    resc = stats_pool.tile([128, slots, C], dt.float32, tag="resc")
                    nc.vector.tensor_mul(resc[:], pmin[:], rcp[:])
                    nc.vector.tensor_scalar_min(resc[:], resc[:], 1.0)

                    nc.vector.tensor_mul(
                        ptpsx_all[:, t * slots : (t + 1) * slots], ptps[:], resc[:]
                    )

                    if variant != "novmul":
                        for k in range(slots):
                            resc_b = resc[:][:, k, None, :].to_broadcast([128, T, C])
                            nc.gpsimd.tensor_tensor(
                                out=wf4[:, k], in0=wf4[:, k], in1=resc_b, op=OP.mult
                            )
                    nc.sync.dma_start(
                        wf_out[sl, :].rearrange("(p k) d -> p (k d)", k=slots), wf[:]
                    )

            if variant != "dma":
                nc.sync.dma_start(
                    ptps_out.rearrange("(t p k) c -> p t (k c)", t=ntiles, k=slots),
                    ptpsx_all[:].rearrange("p (t k) c -> p t (k c)", t=ntiles),
                )

    nc.compile()
    return nc


def get_program(n_sh=NSH, tpw=128, reps=1, variant="full"):
    key = (n_sh, tpw, reps, variant)
    if key not in _prog_cache:
        _prog_cache[key] = _build_program(n_sh, tpw, reps, variant)
    return _prog_cache[key]


def make_mask_table(parents_index: np.ndarray) -> np.ndarray:
    """bf16 [CPAR, C*C]: row m, entry (i,j) == 0.0 iff j is a parent of
    channel i under max_channel m; BIG otherwise (pushes j out of the min)."""
    import ml_dtypes

    mask = np.full((CPAR, C, C), BIG, dtype=np.float32)
    pj = np.asarray(parents_index)  # [CPAR, C, PPAR]
    valid = pj < C  # j == C is the inf sentinel
    m_i, i_i, p_i = np.nonzero(valid)
    mask[m_i, i_i, pj[m_i, i_i, p_i].astype(np.int64)] = 0.0
    return mask.reshape(CPAR, C * C).astype(ml_dtypes.bfloat16)


def make_in_maps(waveforms, max_channels, parents_index, n_cores=NCORES):
    wf_flat = np.ascontiguousarray(np.asarray(waveforms, dtype=np.float32)).reshape(
        N, D
    )
    idx_i32 = np.asarray(max_channels).astype(np.int32)
    mask_table = make_mask_table(parents_index)
    n_sh = N // n_cores
    in_maps = []
    for k in range(n_cores):
        sl = slice(k * n_sh, (k + 1) * n_sh)
        in_maps.append(
            {
                "wf": wf_flat[sl],
                "mask_table": mask_table,
                "idx": idx_i32[sl],
            }
        )
    return in_maps


def kernel(waveforms, max_channels, parents_index):
    from concourse import bass_utils

    nc = get_program()
    in_maps = make_in_maps(waveforms, max_channels, parents_index)
    res = bass_utils.run_bass_kernel_spmd(nc, in_maps, core_ids=list(range(NCORES)))
    wf_out = np.concatenate([r["wf_out"] for r in res.results], axis=0).reshape(N, T, C)
    ptps_out = np.concatenate([r["ptps_out"] for r in res.results], axis=0)
    return wf_out, ptps_out


# revision 33
# speedup vs baseline: 2.2472x; 2.2472x over previous
"""EnforceDecrease kernel for Trainium2 (8 NeuronCores, data-parallel over N).

Computation (per waveform n, channel i):
    ptp[n,i]   = max_t wf[n,t,i] - min_t wf[n,t,i]
    pmin[n,i]  = min over parents p of padded ptp[n, parents_index[mc[n],i,p]]
                 (index c==40 is an +inf sentinel)
    resc[n,i]  = min(pmin/ptp, 1)
    out_wf     = wf * resc;  out_ptp = ptp * resc

Device mapping: partition = waveform (tpw=128 waveforms per tile), DVE does
the T-axis min/max reductions and the masked parent-min reduce, the masked
add (mask u8 * 1e30 + ptp broadcast) is split DVE/GPSIMD for engine balance,
GPSIMD does the big broadcast multiply, HWDGE DMAs stream waveforms in/out,
and per-waveform mask rows are gathered up-front with indirect DMA.
"""

import contextlib

import numpy as np

N, T, C = 8192, 121, 40  # waveforms, timesteps, channels
CPAR, PPAR = 384, 12  # parents table dims
NCORES = 8
NSH = N // NCORES  # waveforms per core
D = T * C
BIG = 1e30

# tiles whose masked-add runs on gpsimd (rest on DVE): engine balance
STT_ON_POOL = 5

_prog_cache: dict = {}


def _build_program(n_sh=NSH, tpw=128, reps=1, variant="full"):
    """Build + compile the per-core Bass program. tpw = waveforms per tile.
    reps > 1 wraps the tile loop in a hardware loop (benchmarking only).
    variant: "full" | "dma" (stream only) | "novmul" (skip pool multiply)."""
    import concourse.bass as bass
    import concourse.tile as tile
    from concourse import bacc, mybir

    dt = mybir.dt
    AX = mybir.AxisListType
    OP = mybir.AluOpType

    slots = tpw // 128  # waveforms per partition per tile
    ntiles = n_sh // tpw
    nslots = ntiles * slots
    mrow = C * C  # mask row elements per waveform

    nc = bacc.Bacc("TRN2", target_bir_lowering=False, debug=False)
    wf_in = nc.dram_tensor("wf", (n_sh, D), dt.float32, kind="ExternalInput").ap()
    mask_table = nc.dram_tensor(
        "mask_table", (CPAR, mrow), dt.bfloat16, kind="ExternalInput"
    ).ap()
    idx = nc.dram_tensor("idx", (n_sh,), dt.int32, kind="ExternalInput").ap()
    wf_out = nc.dram_tensor("wf_out", (n_sh, D), dt.float32, kind="ExternalOutput").ap()
    ptps_out = nc.dram_tensor(
        "ptps_out", (n_sh, C), dt.float32, kind="ExternalOutput"
    ).ap()

    wf_bufs = 7 if slots == 1 else 3
    with tile.TileContext(nc) as tc:
        with (
            tc.tile_pool(name="wfp", bufs=wf_bufs) as wf_pool,
            tc.tile_pool(name="masks", bufs=1) as masks_pool,
            tc.tile_pool(name="tmp", bufs=2 if slots > 1 else 3) as tmp_pool,
            tc.tile_pool(name="stats", bufs=6 if slots == 1 else 3) as stats_pool,
        ):
            # All per-waveform mask rows gathered up-front (gpsimd/SWDGE).
            idx_t = masks_pool.tile([128, ntiles, slots], dt.int32)
            nc.sync.dma_start(
                idx_t[:], idx.rearrange("(t p k) -> p t k", t=ntiles, k=slots)
            )
            masks = masks_pool.tile([128, nslots * mrow], dt.bfloat16)
            ptpsx_all = masks_pool.tile([128, nslots, C], dt.float32)
            for t in range(ntiles):
                for k in range(slots):
                    s = t * slots + k
                    nc.gpsimd.indirect_dma_start(
                        out=masks[:, s * mrow : (s + 1) * mrow],
                        out_offset=None,
                        in_=mask_table,
                        in_offset=bass.IndirectOffsetOnAxis(
                            ap=idx_t[:, t, k : k + 1], axis=0
                        ),
                    )

            rep_ctx = tc.For_i(0, reps, 1) if reps > 1 else contextlib.nullcontext()
            with rep_ctx:
                for t in range(ntiles):
                    sl = slice(t * tpw, (t + 1) * tpw)
                    wf = wf_pool.tile([128, slots * D], dt.float32)
                    nc.sync.dma_start(
                        wf[:], wf_in[sl, :].rearrange("(p k) d -> p (k d)", k=slots)
                    )
                    if variant == "dma":
                        nc.sync.dma_start(
                            wf_out[sl, :].rearrange("(p k) d -> p (k d)", k=slots),
                            wf[:],
                        )
                        continue
                    wf4 = wf[:].rearrange("p (k t c) -> p k t c", k=slots, t=T, c=C)
                    wfT = wf4.transpose([0, 1, 3, 2])  # [128, slots, C, T]

                    mx = stats_pool.tile([128, slots, C], dt.float32, tag="mx")
                    mn = stats_pool.tile([128, slots, C], dt.float32, tag="mn")
                    ptps = stats_pool.tile([128, slots, C], dt.float32, tag="ptps")
                    if variant == "nored":
                        # fake ptps: cheap copy of the first C elements
                        nc.vector.tensor_scalar_max(
                            ptps[:], wf[:].rearrange("p (k d) -> p k d", k=slots)[:, :, :C], 0.01
                        )
                    elif variant == "redc":
                        # timing probe: contiguous-axis reduce (wrong math)
                        wfC = wf[:].rearrange(
                            "p (k c t) -> p k c t", k=slots, c=C, t=T
                        )
                        nc.vector.tensor_reduce(mx[:], wfC, axis=AX.X, op=OP.max)
                        nc.vector.tensor_reduce(mn[:], wfC, axis=AX.X, op=OP.min)
                        nc.vector.tensor_sub(ptps[:], mx[:], mn[:])
                    else:
                        nc.vector.tensor_reduce(mx[:], wfT, axis=AX.X, op=OP.max)
                        nc.vector.tensor_reduce(mn[:], wfT, axis=AX.X, op=OP.min)
                        nc.vector.tensor_sub(ptps[:], mx[:], mn[:])
                    if variant in ("reduces", "redc"):
                        nc.sync.dma_start(
                            wf_out[sl, :].rearrange("(p k) d -> p (k d)", k=slots),
                            wf[:],
                        )
                        continue

                    # masked[p,k,i,j] = mask{0,BIG} + ptp[p,k,j]; then min over j
                    # (TensorTensor APs must be <=3D: one op per slot)
                    masked = tmp_pool.tile([128, slots, C, C], dt.float32, tag="masked")
                    mview = masks[
                        :, t * slots * mrow : (t + 1) * slots * mrow
                    ].rearrange("p (k i j) -> p k i j", k=slots, i=C, j=C)
                    stt_eng = nc.gpsimd if t < STT_ON_POOL else nc.vector
                    for k in range(slots):
                        stt_eng.tensor_tensor(
                            out=masked[:, k],
                            in0=mview[:, k],
                            in1=ptps[:][:, k, None, :].to_broadcast([128, C, C]),
                            op=OP.add,
                        )
                    pmin = stats_pool.tile([128, slots, C], dt.float32, tag="pmin")
                    nc.vector.tensor_reduce(pmin[:], masked[:], axis=AX.X, op=OP.min)

                    rcp = stats_pool.tile([128, slots, C], dt.float32, tag="rcp")
                    nc.vector.reciprocal(rcp[:], ptps[:])
                    resc = stats_pool.tile([128, slots, C], dt.float32, tag="resc")
                    nc.vector.tensor_mul(resc[:], pmin[:], rcp[:])
                    nc.vector.tensor_scalar_min(resc[:], resc[:], 1.0)

                    nc.vector.tensor_mul(
                        ptpsx_all[:, t * slots : (t + 1) * slots], ptps[:], resc[:]
                    )

                    if variant != "novmul":
                        for k in range(slots):
                            resc_b = resc[:][:, k, None, :].to_broadcast([128, T, C])
                            nc.gpsimd.tensor_tensor(
                                out=wf4[:, k], in0=wf4[:, k], in1=resc_b, op=OP.mult
                            )
                    nc.sync.dma_start(
                        wf_out[sl, :].rearrange("(p k) d -> p (k d)", k=slots), wf[:]
                    )

            if variant not in ("dma", "reduces", "redc"):
                nc.sync.dma_start(
                    ptps_out.rearrange("(t p k) c -> p t (k c)", t=ntiles, k=slots),
                    ptpsx_all[:].rearrange("p (t k) c -> p t (k c)", t=ntiles),
                )

    nc.compile()
    return nc


def get_program(n_sh=NSH, tpw=128, reps=1, variant="full"):
    key = (n_sh, tpw, reps, variant)
    if key not in _prog_cache:
        _prog_cache[key] = _build_program(n_sh, tpw, reps, variant)
    return _prog_cache[key]


def make_mask_table(parents_index: np.ndarray) -> np.ndarray:
    """bf16 [CPAR, C*C]: row m, entry (i,j) == 0.0 iff j is a parent of
    channel i under max_channel m; BIG otherwise (pushes j out of the min)."""
    import ml_dtypes

    mask = np.full((CPAR, C, C), BIG, dtype=np.float32)
    pj = np.asarray(parents_index)  # [CPAR, C, PPAR]
    valid = pj < C  # j == C is the inf sentinel
    m_i, i_i, p_i = np.nonzero(valid)
    mask[m_i, i_i, pj[m_i, i_i, p_i].astype(np.int64)] = 0.0
    return mask.reshape(CPAR, C * C).astype(ml_dtypes.bfloat16)


def make_in_maps(waveforms, max_channels, parents_index, n_cores=NCORES):
    wf_flat = np.ascontiguousarray(np.asarray(waveforms, dtype=np.float32)).reshape(
        N, D
    )
    idx_i32 = np.asarray(max_channels).astype(np.int32)
    mask_table = make_mask_table(parents_index)
    n_sh = N // n_cores
    in_maps = []
    for k in range(n_cores):
        sl = slice(k * n_sh, (k + 1) * n_sh)
        in_maps.append(
            {
                "wf": wf_flat[sl],
                "mask_table": mask_table,
                "idx": idx_i32[sl],
            }
        )
    return in_maps


def kernel(waveforms, max_channels, parents_index):
    from concourse import bass_utils

    nc = get_program()
    in_maps = make_in_maps(waveforms, max_channels, parents_index)
    res = bass_utils.run_bass_kernel_spmd(nc, in_maps, core_ids=list(range(NCORES)))
    wf_out = np.concatenate([r["wf_out"] for r in res.results], axis=0).reshape(N, T, C)
    ptps_out = np.concatenate([r["ptps_out"] for r in res.results], axis=0)
    return wf_out, ptps_out
